# revision 1
# baseline (speedup 1.0000x reference)
"""Expert-parallel MoE MLP (ExpertMLP) Bass kernel for 8 Trainium2 NeuronCores.

Problem: x[32,4096,256] @ w_fc[32,256,1024] -> gelu(erf) -> @ w_proj[32,1024,256].

Sharding: expert-parallel. Each of the 8 cores gets 4 experts (slices of the
leading axis of every tensor); no cross-core communication. Inside a core, per
expert e:

  1. x[e] ([4096,256], capacity-major) is transposed on the PE (identity
     matmul, 128x128 blocks) into xT [d, c] so the d-contraction of the first
     matmul lies on the partition axis.
  2. MM1: hT[h_tile, c_chunk] += w_fc_tile.T @ xT_chunk - w_fc's natural
     [d, h] layout is the stationary operand, so it needs no transpose.
  3. GELU (exact erf form) runs on the ACT engine as the PSUM->SBUF eviction.
  4. MM2 uses hT slices as the *stationary* operand and w_proj's natural
     [h, d] layout as the moving operand: out[c_sub, d] += hT_slice.T @
     w_proj_tile. The result lands directly in [capacity, d] orientation, so
     no output transpose is needed.

All matmul operands are float32r (e8m11, 1 PE cycle/row at N>=256 vs 4 for
fp32); producers (DVE copies / ACT gelu) write f32r tiles, which performs the
required rounding. PSUM accumulation stays fp32.
"""

import numpy as np
from contextlib import ExitStack

import bass_rust as _br
import concourse.bass as bass
import concourse.tile as tile
from concourse import mybir
from concourse.bass_utils import run_bass_kernel_spmd
from concourse.masks import make_identity

E, CAP, D, H = 32, 4096, 256, 1024
N_CORES = 8
E_PER = E // N_CORES  # 4 experts per core
P = 128
F32 = mybir.dt.float32
F32R = mybir.dt.float32r
BF16 = mybir.dt.bfloat16

KD = D // P        # 2 k-tiles in MM1's contraction
KH = H // P        # 8 k-tiles in MM2's contraction
NC_CHUNK = 512     # capacity chunk processed per MM1/MM2 round
N_CHUNKS = CAP // NC_CHUNK
H_TILES = H // P
C_TILES = CAP // P


def _fix_waits(nc):
    """walrus here accepts only one sync wait per instruction; hoist excess
    waits onto standalone EventSemaphore instructions inserted before the
    offender (same engine => same sequencer order)."""
    for fn in nc.m.functions:
        for bb in fn.blocks:
            new = []
            changed = False
            for inst in bb.instructions:
                si = inst.sync_info
                if si is not None and len(si.on_wait) > 1:
                    waits = list(si.on_wait)
                    for w in waits[:-1]:
                        ev = mybir.InstEventSemaphore(
                            name=nc.get_next_instruction_name()
                        )
                        ev.engine = inst.engine
                        ev.sync_info = _br.SyncInfo(on_wait=[w], on_update=[])
                        nc.register_instruction(ev)
                        new.append(ev)
                    inst.sync_info = _br.SyncInfo(
                        on_wait=waits[-1:], on_update=list(si.on_update)
                    )
                    changed = True
                new.append(inst)
            if changed:
                bb.instructions = new


def _build():
    nc = bass.Bass(trn_type="TRN2", target_bir_lowering=False, debug=False)
    x = nc.dram_tensor("x", [E_PER, CAP, D], F32, kind="ExternalInput").ap()
    w_fc = nc.dram_tensor("w_fc", [E_PER, D, H], F32, kind="ExternalInput").ap()
    w_proj = nc.dram_tensor("w_proj", [E_PER, H, D], F32, kind="ExternalInput").ap()
    out = nc.dram_tensor("out", [E_PER, CAP, D], F32, kind="ExternalOutput").ap()
    # bf16 staging copies of x so the XBar DMA-transpose (2-byte dtype only)
    # can build xT without burning TensorE cycles on identity transposes.
    # One DRAM tensor per (expert, half): DRAM dependency tracking is
    # tensor-granular, so finer tensors let each transpose start as soon as
    # its own cast chunk lands instead of after all casts.
    CASTCH = CAP // 2  # cast-DMA chunk (rows)
    xbf = [
        [
            nc.dram_tensor(f"xbf{e}_{hh}", [CASTCH, D], BF16).ap()
            for hh in range(CAP // CASTCH)
        ]
        for e in range(E_PER)
    ]

    with tile.TileContext(nc) as tc, ExitStack() as ctx:
        xtp = ctx.enter_context(tc.tile_pool(name="xtp", bufs=2 * E_PER))
        wload = ctx.enter_context(tc.tile_pool(name="wload", bufs=2))
        wfc_p = ctx.enter_context(tc.tile_pool(name="wfc", bufs=2))
        wproj_p = ctx.enter_context(tc.tile_pool(name="wproj", bufs=2))
        ht_p = ctx.enter_context(tc.tile_pool(name="ht", bufs=8))
        out_p = ctx.enter_context(tc.tile_pool(name="outp", bufs=3))
        ps_h = ctx.enter_context(tc.tile_pool(name="ps_h", bufs=2, space="PSUM"))
        ps_o = ctx.enter_context(tc.tile_pool(name="ps_o", bufs=4, space="PSUM"))

        HPACK = 2          # h_tiles packed per PSUM tile / GELU call
        SLAB = 1024        # DMA-transpose slab (capacity columns)

        def load_weights(e):
            wfc_raw = wload.tile([P, KD, H], F32, tag="wl")
            nc.sync.dma_start(wfc_raw[:], w_fc[e].rearrange("(k p) h -> p k h", p=P))
            wfc = wfc_p.tile([P, KD, H], BF16, tag="wfc")
            nc.vector.tensor_copy(wfc[:], wfc_raw[:])
            wproj_raw = wload.tile([P, KH, D], F32, tag="wl")
            nc.sync.dma_start(
                wproj_raw[:], w_proj[e].rearrange("(k p) d -> p k d", p=P)
            )
            wproj = wproj_p.tile([P, KH, D], BF16, tag="wproj")
            nc.vector.tensor_copy(wproj[:], wproj_raw[:])
            return wfc, wproj

        # ---- prologue: expert 0's weights first, then stage all experts' xT:
        # DRAM->DRAM cast x[e]->bf16 in half-chunks (q0 FIFO => e0 first),
        # then XBar-transpose 1024-column slabs into SBUF on the scalar HWDGE
        # queue so they don't queue behind weight/output traffic on q1.
        # MM1 of (e, chunk) only needs its slab, so compute starts early.
        w0 = load_weights(0)
        for e in range(E_PER):
            for hh in range(CAP // CASTCH):
                rs = slice(hh * CASTCH, (hh + 1) * CASTCH)
                nc.gpsimd.dma_start(xbf[e][hh][:], x[e][rs])
        SPH = CASTCH // SLAB  # slabs per cast half
        xts = []
        for e in range(E_PER):
            xt = [
                [
                    xtp.tile([P, SLAB], BF16, tag="xt", name=f"xt{e}_{k}_{s}")
                    for s in range(CAP // SLAB)
                ]
                for k in range(KD)
            ]
            for s in range(CAP // SLAB):
                ls = slice((s % SPH) * SLAB, (s % SPH + 1) * SLAB)
                for k in range(KD):
                    nc.sync.dma_start_transpose(
                        xt[k][s][:], xbf[e][s // SPH][ls, k * P:(k + 1) * P]
                    )
            xts.append(xt)

        for e in range(E_PER):
            xt = xts[e]
            wfc, wproj = w0 if e == 0 else load_weights(e)

            # ---- MM1 -> GELU -> MM2 per capacity chunk ----
            # MM1 accumulates HPACK h_tiles into one multi-bank PSUM tile so
            # GELU evicts in wider (cheaper) ACTIVATE calls; hT is written in
            # bf16 so MM2's per-matmul weight loads run at 2-byte FWL speed.
            for nci in range(N_CHUNKS):
                csl = slice(nci * NC_CHUNK, (nci + 1) * NC_CHUNK)
                ht_tiles = []  # HPACK-wide bf16 tiles
                for hp in range(H_TILES // HPACK):
                    psh = ps_h.tile([P, HPACK, NC_CHUNK], F32, tag="psh")
                    for j in range(HPACK):
                        hi = hp * HPACK + j
                        for k in range(KD):
                            sidx = (nci * NC_CHUNK) // SLAB
                            soff = (nci * NC_CHUNK) % SLAB
                            nc.tensor.matmul(
                                psh[:, j, :],
                                wfc[:, k, hi * P:(hi + 1) * P],
                                xt[k][sidx][:, soff:soff + NC_CHUNK],
                                start=(k == 0),
                                stop=(k == KD - 1),
                            )
                    ht = ht_p.tile([P, HPACK, NC_CHUNK], BF16, tag="ht")
                    nc.scalar.activation(
                        ht[:], psh[:], mybir.ActivationFunctionType.Gelu
                    )
                    ht_tiles.append(ht)

                ob = out_p.tile([P, NC_CHUNK // P, D], F32, tag="ob")
                for s in range(NC_CHUNK // P):
                    pso = ps_o.tile([P, D], F32, tag="pso")
                    for k in range(KH):
                        nc.tensor.matmul(
                            pso[:],
                            ht_tiles[k // HPACK][:, k % HPACK, s * P:(s + 1) * P],
                            wproj[:, k, :],
                            start=(k == 0),
                            stop=(k == KH - 1),
                        )
                    nc.vector.tensor_copy(ob[:, s, :], pso[:])
                nc.sync.dma_start(
                    out[e, csl, :].rearrange("(s p) d -> p s d", p=P), ob[:]
                )

    _fix_waits(nc)
    return nc


_CACHE = {}


def _get_nc():
    if "nc" not in _CACHE:
        _CACHE["nc"] = _build()
    return _CACHE["nc"]


def kernel(x, w_fc, w_proj, trace=False):
    assert x.shape == (E, CAP, D) and w_fc.shape == (E, D, H)
    assert w_proj.shape == (E, H, D)
    nc = _get_nc()
    x = np.ascontiguousarray(x, dtype=np.float32)
    w_fc = np.ascontiguousarray(w_fc, dtype=np.float32)
    w_proj = np.ascontiguousarray(w_proj, dtype=np.float32)
    in_maps = [
        {
            "x": x[i * E_PER:(i + 1) * E_PER],
            "w_fc": w_fc[i * E_PER:(i + 1) * E_PER],
            "w_proj": w_proj[i * E_PER:(i + 1) * E_PER],
        }
        for i in range(N_CORES)
    ]
    res = run_bass_kernel_spmd(nc, in_maps, list(range(N_CORES)), trace=trace)
    out = np.concatenate([r["out"] for r in res.results], axis=0)
    if trace:
        kernel.last_results = res
    return out



# revision 3
# speedup vs baseline: 1.1595x; 1.1595x over previous
"""Expert-parallel MoE MLP (ExpertMLP) Bass kernel for 8 Trainium2 NeuronCores.

Problem: x[32,4096,256] @ w_fc[32,256,1024] -> gelu(erf) -> @ w_proj[32,1024,256].

Sharding: expert-parallel, 4 experts per core, no cross-core communication.

Per-core dataflow (v2 — software-pipelined):

  * x[e] is staged to bf16 in DRAM slab-by-slab (1024 rows per SWDGE cast-DMA)
    and immediately XBar-DMA-transposed into SBUF as xT[d, c] slabs, so the
    first MM1 can start ~10us in instead of ~55us (v1 staged everything
    up front and serialized on the Sync queue).
  * MM1: hT[h_tile, c] += w_fc[d,h_tile].T @ xT[d, c-chunk]; GELU (ACT) evicts
    PSUM->SBUF in bf16.
  * MM2: out[c_sub, d] += hT_slice.T @ w_proj[h, d], PSUM -> DVE copy -> DMA.
  * The PE instruction stream interleaves [MM2 of chunk g-1] with [MM1 of
    chunk g] so the GELU chain (5.3us per chunk on ACT) has a full chunk of
    slack and never stalls the PE (v1 lost ~2.2us per chunk to it).
  * ~24 dummy matmuls on a zeroed tile run at t~0 to open the HAM clock gate
    (PE idles at 1.2 GHz otherwise) while DMAs stage the first slabs.
  * Weights and x slabs for expert e+1 prefetch across expert e's chunks.

All matmul operands are bf16 (PSUM accumulation stays fp32); measured rel err
vs the f32 reference ~4e-3 (tolerance 2e-2).
"""

import numpy as np
from contextlib import ExitStack

import bass_rust as _br
import concourse.bass as bass
import concourse.tile as tile
from concourse import mybir
from concourse.bass_utils import run_bass_kernel_spmd

E, CAP, D, H = 32, 4096, 256, 1024
N_CORES = 8
E_PER = E // N_CORES  # 4 experts per core
P = 128
F32 = mybir.dt.float32
BF16 = mybir.dt.bfloat16

KD = D // P            # 2 k-tiles in MM1's contraction
KH = H // P            # 8 k-tiles in MM2's contraction
NC_CHUNK = 512         # capacity chunk per pipeline block
CHUNKS_PER_E = CAP // NC_CHUNK   # 8
SLAB = 1024            # x staging slab (capacity rows per cast / transpose)
SLABS_PER_E = CAP // SLAB        # 4
HPACK = 2              # h_tiles packed per PSUM tile / GELU call
HGROUPS = H // P // HPACK        # 4
G_TOTAL = E_PER * CHUNKS_PER_E   # 32 pipeline blocks of real work


def _fix_waits(nc):
    """walrus here accepts only one sync wait per instruction; hoist excess
    waits onto standalone EventSemaphore instructions inserted before the
    offender (same engine => same sequencer order)."""
    for fn in nc.m.functions:
        for bb in fn.blocks:
            new = []
            changed = False
            for inst in bb.instructions:
                si = inst.sync_info
                if si is not None and len(si.on_wait) > 1:
                    waits = list(si.on_wait)
                    for w in waits[:-1]:
                        ev = mybir.InstEventSemaphore(
                            name=nc.get_next_instruction_name()
                        )
                        ev.engine = inst.engine
                        ev.sync_info = _br.SyncInfo(on_wait=[w], on_update=[])
                        nc.register_instruction(ev)
                        new.append(ev)
                    inst.sync_info = _br.SyncInfo(
                        on_wait=waits[-1:], on_update=list(si.on_update)
                    )
                    changed = True
                new.append(inst)
            if changed:
                bb.instructions = new


def _build():
    nc = bass.Bass(trn_type="TRN2", target_bir_lowering=False, debug=False)
    x = nc.dram_tensor("x", [E_PER, CAP, D], F32, kind="ExternalInput").ap()
    w_fc = nc.dram_tensor("w_fc", [E_PER, D, H], F32, kind="ExternalInput").ap()
    w_proj = nc.dram_tensor("w_proj", [E_PER, H, D], F32, kind="ExternalInput").ap()
    out = nc.dram_tensor("out", [E_PER, CAP, D], F32, kind="ExternalOutput").ap()
    # bf16 staging of x, one DRAM tensor per (expert, slab): DRAM dependency
    # tracking is tensor-granular, so each XBar transpose starts as soon as
    # its own slab's cast lands.
    xbf = [
        [
            nc.dram_tensor(f"xbf{e}_{s}", [SLAB, D], BF16).ap()
            for s in range(SLABS_PER_E)
        ]
        for e in range(E_PER)
    ]

    with tile.TileContext(nc) as tc, ExitStack() as ctx:
        xtp = ctx.enter_context(tc.tile_pool(name="xtp", bufs=2 * SLABS_PER_E * KD))
        wload = ctx.enter_context(tc.tile_pool(name="wload", bufs=2))
        wfc_p = ctx.enter_context(tc.tile_pool(name="wfc", bufs=2))
        wproj_p = ctx.enter_context(tc.tile_pool(name="wproj", bufs=2))
        ht_p = ctx.enter_context(tc.tile_pool(name="ht", bufs=2 * HGROUPS))
        out_p = ctx.enter_context(tc.tile_pool(name="outp", bufs=3))
        misc_p = ctx.enter_context(tc.tile_pool(name="misc", bufs=1))
        ps_h = ctx.enter_context(tc.tile_pool(name="ps_h", bufs=2, space="PSUM"))
        ps_o = ctx.enter_context(tc.tile_pool(name="ps_o", bufs=4, space="PSUM"))

        # ---- PE warmup: the HAM clock gate keeps the PE at 1.2 GHz until it
        # has seen ~3.4us of sustained activity. Dummy matmuls on a zeroed
        # tile warm it while the first slab casts/transposes are in flight.
        wu = misc_p.tile([P, P], BF16, tag="wu")
        nc.vector.memset(wu[:], 0.0)
        wps = ps_o.tile([P, D], F32, tag="pso")
        for _ in range(24):
            nc.tensor.matmul(wps[:, 0:P], wu[:], wu[:], start=True, stop=True)

        xts = [[[None] * SLABS_PER_E for _ in range(KD)] for _ in range(E_PER)]

        def issue_cast(e, s):
            rs = slice(s * SLAB, (s + 1) * SLAB)
            nc.gpsimd.dma_start(xbf[e][s][:], x[e][rs])

        def issue_transpose(e, s):
            for k in range(KD):
                t = xtp.tile([P, SLAB], BF16, tag="xt", name=f"xt{e}_{k}_{s}")
                nc.sync.dma_start_transpose(t[:], xbf[e][s][:, k * P:(k + 1) * P])
                xts[e][k][s] = t

        def load_wfc_raw(e):
            raw = wload.tile([P, KD, H], F32, tag="wl")
            nc.sync.dma_start(raw[:], w_fc[e].rearrange("(k p) h -> p k h", p=P))
            return raw

        def cast_wfc(raw):
            t = wfc_p.tile([P, KD, H], BF16, tag="wfc")
            nc.vector.tensor_copy(t[:], raw[:])
            return t

        def load_wproj_raw(e):
            raw = wload.tile([P, KH, D], F32, tag="wl")
            nc.sync.dma_start(raw[:], w_proj[e].rearrange("(k p) d -> p k d", p=P))
            return raw

        def cast_wproj(raw):
            t = wproj_p.tile([P, KH, D], BF16, tag="wproj")
            nc.vector.tensor_copy(t[:], raw[:])
            return t

        # ---- prologue: stage expert 0 (weights first on the Sync queue so
        # the transposes' semaphore waits don't delay them).
        wfcs = [None] * E_PER
        wprojs = [None] * E_PER
        issue_cast(0, 0)
        r = load_wfc_raw(0)
        wfcs[0] = cast_wfc(r)
        r = load_wproj_raw(0)
        wprojs[0] = cast_wproj(r)
        issue_transpose(0, 0)
        for s in range(1, SLABS_PER_E):
            issue_cast(0, s)
            issue_transpose(0, s)

        ht_all = {}
        pso_cur = None   # pso tiles of the chunk whose MM2 runs next block
        wraw = None

        for g in range(G_TOTAL + 1):
            e, i = divmod(g, CHUNKS_PER_E)
            ep, ip = divmod(g - 1, CHUNKS_PER_E)  # chunk drained this block

            # ---- prefetch schedule for expert e+1 ----
            if g < G_TOTAL and e + 1 < E_PER:
                if i in (0, 2, 4, 6):
                    issue_cast(e + 1, i // 2)
                elif i in (1, 3, 5, 7):
                    issue_transpose(e + 1, (i - 1) // 2)
                if i == 0:
                    wraw = load_wfc_raw(e + 1)
                elif i == 3:
                    wfcs[e + 1] = cast_wfc(wraw)
                elif i == 4:
                    wraw = load_wproj_raw(e + 1)
                elif i == 7:
                    wprojs[e + 1] = cast_wproj(wraw)

            # ---- interleaved PE stream: MM2(g-1) + MM1(g) ----
            for hp in range(HGROUPS):
                if g >= 1:
                    ht = ht_all[(g - 1, hp)]
                    wp = wprojs[ep]
                    for s in range(NC_CHUNK // P):
                        for j in range(HPACK):
                            kk = hp * HPACK + j
                            nc.tensor.matmul(
                                pso_cur[s][:],
                                ht[:, j, s * P:(s + 1) * P],
                                wp[:, kk, :],
                                start=(kk == 0),
                                stop=(kk == KH - 1),
                            )
                if g < G_TOTAL:
                    psh = ps_h.tile([P, HPACK, NC_CHUNK], F32, tag="psh")
                    sidx, soff = i // 2, (i % 2) * NC_CHUNK
                    for j in range(HPACK):
                        hi = hp * HPACK + j
                        for k in range(KD):
                            nc.tensor.matmul(
                                psh[:, j, :],
                                wfcs[e][:, k, hi * P:(hi + 1) * P],
                                xts[e][k][sidx][:, soff:soff + NC_CHUNK],
                                start=(k == 0),
                                stop=(k == KD - 1),
                            )
                    ht = ht_p.tile([P, HPACK, NC_CHUNK], BF16, tag="ht")
                    nc.scalar.activation(
                        ht[:], psh[:], mybir.ActivationFunctionType.Gelu
                    )
                    ht_all[(g, hp)] = ht

            # ---- drain chunk g-1: PSUM -> SBUF -> DRAM ----
            if g >= 1:
                csl = slice(ip * NC_CHUNK, (ip + 1) * NC_CHUNK)
                ob = out_p.tile([P, NC_CHUNK // P, D], F32, tag="ob")
                for s in range(NC_CHUNK // P):
                    nc.vector.tensor_copy(ob[:, s, :], pso_cur[s][:])
                nc.sync.dma_start(
                    out[ep, csl, :].rearrange("(s p) d -> p s d", p=P), ob[:]
                )
                for hp in range(HGROUPS):
                    del ht_all[(g - 1, hp)]

            # ---- allocate chunk g's MM2 accumulators for next block ----
            if g < G_TOTAL:
                pso_cur = [
                    ps_o.tile([P, D], F32, tag="pso", name=f"pso{g}_{s}")
                    for s in range(NC_CHUNK // P)
                ]

    _fix_waits(nc)
    return nc


_CACHE = {}


def _get_nc():
    if "nc" not in _CACHE:
        _CACHE["nc"] = _build()
    return _CACHE["nc"]


def kernel(x, w_fc, w_proj, trace=False):
    assert x.shape == (E, CAP, D) and w_fc.shape == (E, D, H)
    assert w_proj.shape == (E, H, D)
    nc = _get_nc()
    x = np.ascontiguousarray(x, dtype=np.float32)
    w_fc = np.ascontiguousarray(w_fc, dtype=np.float32)
    w_proj = np.ascontiguousarray(w_proj, dtype=np.float32)
    in_maps = [
        {
            "x": x[i * E_PER:(i + 1) * E_PER],
            "w_fc": w_fc[i * E_PER:(i + 1) * E_PER],
            "w_proj": w_proj[i * E_PER:(i + 1) * E_PER],
        }
        for i in range(N_CORES)
    ]
    res = run_bass_kernel_spmd(nc, in_maps, list(range(N_CORES)), trace=trace)
    out = np.concatenate([r["out"] for r in res.results], axis=0)
    if trace:
        kernel.last_results = res
    return out
